# revision 1
# baseline (speedup 1.0000x reference)
"""Multi-head attention (B=2, S=2048, D=1024, H=16 heads, causal) on 8 TRN2 cores.

Sharding: core i handles batch b=i//4 and head group g=i%4 (4 heads = 256 dims).
Each core computes QKV projections for its head group, causal flash-style
attention, and a partial output projection (its 256-dim slice of the
contraction). Host sums the 4 partials per batch and adds the output bias.

On-chip layouts (per core):
  - Q^T, K^T: [n=256, s=2048] (head-pairs stacked on 128 partitions x 2 tiles)
  - V: [s, n] natural, stored per (s-tile, head) as [128, 65] with a ones
    column appended -> the P@V matmul emits the softmax denominator for free
  - scores computed transposed: S^T[k, q] tiles [128, 512]
  - softmax without max-subtraction (scores are O(+-6); exact vs reference
    because softmax is shift-invariant and masked lanes hit exp->0)
  - all matmuls in float32r (full PE rate), fp32 PSUM accumulation
"""
import sys

import numpy as np

try:
    import concourse.bass as bass  # noqa: F401
except ImportError:
    sys.path.insert(0, "/opt/trn_rl_repo")

import concourse.bass as bass
import concourse.mybir as mybir
import concourse.tile as tile
from concourse import bacc
from concourse.bass_utils import run_bass_kernel_spmd

FP32 = mybir.dt.float32
F32R = mybir.dt.float32r
AF = mybir.ActivationFunctionType

B, S, D = 2, 2048, 1024
NH, DK = 16, 64
G = 4              # head groups (cores per batch)
HPG = NH // G      # heads per group = 4
NG = HPG * DK      # dims per group = 256
CH = 512           # q-chunk width
NCH = S // CH      # 4 chunks
NKT = S // 128     # 16 k-tiles
SCALE = 1.0 / np.sqrt(DK)

TRACE = False          # test harness can set kernel.TRACE = True
LAST_RESULTS = None    # test harness reads kernel.LAST_RESULTS

_NC_CACHE = None


def _build_nc():
    nc = bacc.Bacc()
    xqT = nc.declare_dram_parameter("xqT", [D, S], FP32, isOutput=False)
    xkT = nc.declare_dram_parameter("xkT", [D, S], FP32, isOutput=False)
    xvT = nc.declare_dram_parameter("xvT", [D, S], FP32, isOutput=False)
    wq = nc.declare_dram_parameter("wq", [D, NG], FP32, isOutput=False)
    wk = nc.declare_dram_parameter("wk", [D, NG], FP32, isOutput=False)
    wv = nc.declare_dram_parameter("wv", [D, NG], FP32, isOutput=False)
    wo = nc.declare_dram_parameter("wo", [NG, D], FP32, isOutput=False)
    bq = nc.declare_dram_parameter("bq", [128, 2], FP32, isOutput=False)
    bk = nc.declare_dram_parameter("bk", [128, 2], FP32, isOutput=False)
    bv = nc.declare_dram_parameter("bv", [128, 2], FP32, isOutput=False)
    mstrip = nc.declare_dram_parameter("mstrip", [128, 512], FP32, isOutput=False)
    onesd = nc.declare_dram_parameter("onesd", [128, 64], FP32, isOutput=False)
    out = nc.declare_dram_parameter("out", [S, D], FP32, isOutput=True)

    KD = D // 128  # 8 contraction tiles for projections

    with tile.TileContext(nc) as tc:
        with (
            tc.tile_pool(name="wpool", bufs=1) as wpool,
            tc.tile_pool(name="cpool", bufs=1) as cpool,
            tc.tile_pool(name="big", bufs=1) as big,
            tc.tile_pool(name="xq", bufs=3) as xqp,
            tc.tile_pool(name="xk", bufs=3) as xkp,
            tc.tile_pool(name="xv", bufs=2) as xvp,
            tc.tile_pool(name="pp", bufs=2) as ppool,
            tc.tile_pool(name="sm", bufs=2) as smp,
            tc.tile_pool(name="ost", bufs=2) as ostp,
            tc.tile_pool(name="ps_proj", bufs=1, space="PSUM") as ps_proj,
            tc.tile_pool(name="ps_s", bufs=2, space="PSUM") as ps_s,
            tc.tile_pool(name="ps_av", bufs=1, space="PSUM") as ps_av,
            tc.tile_pool(name="ps_o", bufs=1, space="PSUM") as ps_o,
        ):
            # ---- constants / weights (resident) ----
            wq_sb = wpool.tile([128, KD * NG], F32R, tag="wq")
            wk_sb = wpool.tile([128, KD * NG], F32R, tag="wk")
            wv_sb = wpool.tile([128, KD * NG], F32R, tag="wv")
            wo_sb = wpool.tile([128, 2 * D], F32R, tag="wo")
            for w_sb, w_dram in ((wq_sb, wq), (wk_sb, wk), (wv_sb, wv)):
                nc.sync.dma_start(
                    out=w_sb[:].rearrange("p (k n) -> p k n", k=KD),
                    in_=w_dram.rearrange("(k p) n -> p k n", p=128).bitcast(F32R),
                )
            nc.sync.dma_start(
                out=wo_sb[:].rearrange("p (k m) -> p k m", k=2),
                in_=wo.rearrange("(k p) m -> p k m", p=128).bitcast(F32R),
            )
            bq_sb = cpool.tile([128, 2], FP32, tag="bq")
            bk_sb = cpool.tile([128, 2], FP32, tag="bk")
            bv_sb = cpool.tile([128, 2], FP32, tag="bv")
            nc.sync.dma_start(out=bq_sb[:], in_=bq[:])
            nc.sync.dma_start(out=bk_sb[:], in_=bk[:])
            nc.sync.dma_start(out=bv_sb[:], in_=bv[:])
            mask_sb = cpool.tile([128, 512], FP32, tag="mask")
            nc.sync.dma_start(out=mask_sb[:], in_=mstrip[:])
            ones64 = cpool.tile([1, 64], F32R, tag="ones64")
            nc.sync.dma_start(out=ones64[:], in_=onesd[0:1, :].bitcast(F32R))

            # ---- persistent activations ----
            q_sb = [big.tile([128, S], F32R, tag=f"q{m}", name=f"q{m}") for m in range(2)]
            k_sb = [big.tile([128, S], F32R, tag=f"k{m}", name=f"k{m}") for m in range(2)]
            ctx_sb = [big.tile([128, S], F32R, tag=f"ctx{m}", name=f"ctx{m}") for m in range(2)]
            # V: 16 s-tiles x 4 heads x (64 + ones)
            v_sb = big.tile([128, NKT * HPG * 65], F32R, tag="v")
            # fill the ones columns (col 64 of every 65-wide head block)
            vview = v_sb[:].rearrange("p (t e) -> p t e", e=65)[:, :, 64:65]
            nc.sync.dma_start(out=vview, in_=onesd[:, :, None].bitcast(F32R))

            for c in range(NCH):
                c0 = c * CH
                # ---- load x^T chunk tiles ----
                xq_t, xk_t, xv_t = [], [], []
                for pool_, dram_, lst in ((xqp, xqT, xq_t), (xkp, xkT, xk_t),
                                          (xvp, xvT, xv_t)):
                    for hh in range(2):
                        t_ = pool_.tile([128, 4 * CH], F32R, tag="x", name="xt")
                        r = slice(hh * 512, hh * 512 + 512)
                        nc.sync.dma_start(
                            out=t_[:].rearrange("p (k s) -> p k s", k=4),
                            in_=dram_[r, c0:c0 + CH]
                                .rearrange("(k p) s -> p k s", p=128)
                                .bitcast(F32R),
                        )
                        lst.append(t_)

                # ---- Q^T / K^T projections: out[n, s] ----
                for (x_t, w_sb_, dst, b_sb_) in (
                    (xq_t, wq_sb, q_sb, bq_sb),
                    (xk_t, wk_sb, k_sb, bk_sb),
                ):
                    for m in range(2):
                        pt = ps_proj.tile([128, CH], FP32, tag="pj", name="pt")
                        for kd in range(KD):
                            nc.tensor.matmul(
                                pt[:],
                                lhsT=w_sb_[:, kd * NG + m * 128: kd * NG + m * 128 + 128],
                                rhs=x_t[kd // 4][:, (kd % 4) * CH:
                                                 (kd % 4) * CH + CH],
                                start=(kd == 0), stop=(kd == KD - 1),
                            )
                        nc.vector.tensor_scalar_add(
                            dst[m][:, c0:c0 + CH], pt[:], b_sb_[:, m:m + 1]
                        )

                # ---- V projection: out[s, n], 2 s-subs per psum tile ----
                for half in range(2):
                    pv = ps_proj.tile([128, CH], FP32, tag="pj", name="pv")
                    for ss in (2 * half, 2 * half + 1):
                        col = (ss - 2 * half) * NG
                        for kd in range(KD):
                            nc.tensor.matmul(
                                pv[:, col:col + NG],
                                lhsT=xv_t[kd // 4][:, (kd % 4) * CH + ss * 128:
                                                   (kd % 4) * CH + ss * 128 + 128],
                                rhs=wv_sb[:, kd * NG: kd * NG + NG],
                                start=(kd == 0), stop=(kd == KD - 1),
                            )
                    for ss in (2 * half, 2 * half + 1):
                        st = 4 * c + ss
                        col = (ss - 2 * half) * NG
                        # [128, 4, 64] -> v_sb block [128, 4, 65][:, :, :64]
                        dst = v_sb[:, st * HPG * 65: (st + 1) * HPG * 65]
                        dst = dst.rearrange("p (h e) -> p h e", h=HPG)[:, :, 0:64]
                        src = pv[:, col:col + NG].rearrange("p (h e) -> p h e", h=HPG)
                        nc.vector.tensor_copy(dst, src)

                # ---- attention for q-chunk c, all 4 heads ----
                # head pairs (0,1) and (2,3): the two heads' score matmuls use
                # contraction rows 0-63 / 64-127 -> distinct PE row-groups ->
                # the array runs them concurrently when issued back-to-back
                for hp in (0, 2):
                    mt = hp // 2
                    pav = [ps_av.tile([128, CH], FP32, tag=f"av{i}", name=f"pav{i}")
                           for i in range(2)]
                    for kt in range(4 * c + 4):
                        j = kt - 4 * c
                        # causal: q-cols < 128j of this chunk are fully masked
                        w = CH - 128 * j if j > 0 else CH
                        qo = c0 + (CH - w)
                        sp = ps_s.tile([128, 2 * CH], FP32, tag="sp", name="sp")
                        for i in range(2):
                            po = i * 64
                            nc.tensor.matmul(
                                sp[:, i * CH: i * CH + w],
                                lhsT=k_sb[mt][po:po + 64, kt * 128: kt * 128 + 128],
                                rhs=q_sb[mt][po:po + 64, qo:qo + w],
                                start=True, stop=True,
                            )
                        pp = ppool.tile([128, 2 * CH], F32R, tag="p", name="pp")
                        sview = sp[:].rearrange("p (t x) -> p t x", t=2)[:, :, 0:w]
                        pview = pp[:].rearrange("p (t x) -> p t x", t=2)[:, :, 0:w]
                        nc.scalar.activation(pview, sview, AF.Exp, scale=SCALE)
                        if j >= 0:
                            nc.vector.tensor_mul(
                                pview, pview,
                                mask_sb[:, None, 0:w].to_broadcast((128, 2, w)),
                            )
                        for i in range(2):
                            h = hp + i
                            vcol = (kt * HPG + h) * 65
                            nc.tensor.matmul(
                                pav[i][0:65, CH - w:CH],
                                lhsT=v_sb[:, vcol:vcol + 65],
                                rhs=pp[:, i * CH: i * CH + w],
                                start=(kt == 0), stop=(kt == 4 * c + 3),
                            )
                    for i in range(2):
                        po = i * 64
                        craw = smp.tile([64, CH], FP32, tag="craw", name="craw")
                        den = smp.tile([1, CH], F32R, tag="den", name="den")
                        nc.vector.tensor_copy(craw[:], pav[i][0:64, :])
                        nc.vector.tensor_copy(den[:], pav[i][64:65, :])
                        pbc = ps_o.tile([64, CH], FP32, tag="o", name="pbc")
                        nc.tensor.matmul(pbc[:], lhsT=ones64[:], rhs=den[:],
                                         start=True, stop=True)
                        rb = smp.tile([64, CH], FP32, tag="rb")
                        nc.vector.reciprocal_approx_fast(out=rb[:], in_=pbc[:])
                        dst = ctx_sb[mt][po:po + 64, c0:c0 + CH]
                        nc.vector.tensor_mul(dst, craw[:], rb[:])
                        nc.vector.tensor_scalar_add(dst, dst,
                                                    bv_sb[po:po + 64, mt:mt + 1])

                # ---- partial output projection for chunk c ----
                for st in range(4):
                    r0 = c0 + st * 128
                    ot = ostp.tile([128, 2 * CH], FP32, tag="ot", name="ot")
                    for mo in range(2):
                        pot = ps_o.tile([128, CH], FP32, tag="o", name="pot")
                        for kk in range(2):
                            nc.tensor.matmul(
                                pot[:],
                                lhsT=ctx_sb[kk][:, r0:r0 + 128],
                                rhs=wo_sb[:, kk * D + mo * CH: kk * D + mo * CH + CH],
                                start=(kk == 0), stop=(kk == 1),
                            )
                        nc.vector.tensor_copy(ot[:, mo * CH: mo * CH + CH], pot[:])
                    nc.sync.dma_start(out=out[r0:r0 + 128, :], in_=ot[:])

    nc.compile()
    return nc


def _get_nc():
    global _NC_CACHE
    if _NC_CACHE is None:
        _NC_CACHE = _build_nc()
    return _NC_CACHE


def _mask_strip() -> np.ndarray:
    # strip[p, y] = 1.0 iff y >= p; with the causal sub-range offset applied
    # to the q-columns, every diagonal k-tile masks with strip[:, 0:w]
    y = np.arange(512)[None, :]
    p = np.arange(128)[:, None]
    return (y >= p).astype(np.float32)


def _reference_fallback(query, key, value, mask, wq, bq, wk, bk, wv, bv, wo, bo):
    out = np.empty((B, S, D), np.float32)
    for b in range(B):
        Q = (query[b] @ wq + bq).reshape(S, NH, DK).transpose(1, 0, 2)
        K = (key[b] @ wk + bk).reshape(S, NH, DK).transpose(1, 0, 2)
        V = (value[b] @ wv + bv).reshape(S, NH, DK).transpose(1, 0, 2)
        sc = np.einsum("hqd,hkd->hqk", Q, K).astype(np.float32) / np.sqrt(DK)
        sc = np.where(mask[b][None] == 0, -1.0e9, sc)
        sc -= sc.max(-1, keepdims=True)
        e = np.exp(sc)
        attn = e / e.sum(-1, keepdims=True)
        ctx = np.einsum("hqk,hkd->hqd", attn, V).transpose(1, 0, 2).reshape(S, D)
        out[b] = ctx @ wo + bo
    return out


def kernel(query, key, value, mask, wq, bq, wk, bk, wv, bv, wo, bo):
    global LAST_RESULTS
    query = np.asarray(query, np.float32)
    key = np.asarray(key, np.float32)
    value = np.asarray(value, np.float32)
    mask = np.asarray(mask)
    wq, bq = np.asarray(wq, np.float32), np.asarray(bq, np.float32)
    wk, bk = np.asarray(wk, np.float32), np.asarray(bk, np.float32)
    wv, bv = np.asarray(wv, np.float32), np.asarray(bv, np.float32)
    wo, bo = np.asarray(wo, np.float32), np.asarray(bo, np.float32)

    tril = np.tril(np.ones((S, S), mask.dtype))
    if not all(np.array_equal(mask[b], tril) for b in range(B)):
        return _reference_fallback(query, key, value, mask, wq, bq, wk, bk,
                                   wv, bv, wo, bo)

    strip = _mask_strip()
    ones_arr = np.ones((128, 64), np.float32)
    xT = {}
    for b in range(B):
        xT[("q", b)] = np.ascontiguousarray(query[b].T)
        xT[("k", b)] = np.ascontiguousarray(key[b].T)
        xT[("v", b)] = np.ascontiguousarray(value[b].T)

    in_maps = []
    for core in range(8):
        b, g = core // G, core % G
        cs = slice(g * NG, (g + 1) * NG)
        in_maps.append({
            "xqT": xT[("q", b)],
            "xkT": xT[("k", b)],
            "xvT": xT[("v", b)],
            "wq": np.ascontiguousarray(wq[:, cs]),
            "wk": np.ascontiguousarray(wk[:, cs]),
            "wv": np.ascontiguousarray(wv[:, cs]),
            "wo": np.ascontiguousarray(wo[cs, :]),
            "bq": np.ascontiguousarray(bq[cs].reshape(2, 128).T),
            "bk": np.ascontiguousarray(bk[cs].reshape(2, 128).T),
            "bv": np.ascontiguousarray(bv[cs].reshape(2, 128).T),
            "mstrip": strip,
            "onesd": ones_arr,
        })

    nc = _get_nc()
    res = run_bass_kernel_spmd(nc, in_maps, list(range(8)), trace=TRACE)
    LAST_RESULTS = res

    out = np.empty((B, S, D), np.float32)
    for b in range(B):
        acc = res.results[b * G]["out"].astype(np.float32)
        for g in range(1, G):
            acc = acc + res.results[b * G + g]["out"]
        out[b] = acc + bo
    return out



# revision 36
# speedup vs baseline: 1.2744x; 1.2744x over previous
"""Multi-head attention (B=2, S=2048, D=1024, H=16 heads, causal) on 8 TRN2 cores.

Sharding: core i handles batch b=i//4 and head group g=i%4 (4 heads = 256 dims).
Each core computes QKV projections for its head group, causal attention, and a
partial output projection (its 256-dim slice of the contraction). Host sums the
4 partials per batch and adds the output bias.

v2 design (all matmuls bf16, fp32 PSUM accumulation):
  - Q^T, K^T: [n=256, s=2048] bf16 (head-pairs stacked on 128 partitions x 2)
  - V natural [s, n] bf16 per (s-tile, pair) as two 128-col lhsT blocks:
      block0 = [V_h0(64) | ones(1) | 0(63)]  -> AV out: ctx_h0 @ parts 0-63,
                                                den_h0 @ part 64
      block1 = [ones(1) | 0(63) | V_h1(64)]  -> AV out: den_h1 @ part 0,
                                                ctx_h1 @ parts 64-127
    so softmax denominators come out of the AV matmul for free AND the
    normalize tensor ops are lane-aligned with their ctx rows.
  - causal mask applied inside the scores PSUM via an extra matmul
    (diag(-1e9) @ strip[128,128]) -- only the first 128 cols of each diagonal
    tile can be masked, so N=128.
  - exp on ACT engine (scale=1/sqrt(dk)), output bf16 directly.
  - normalize: reciprocal_approx_fast from PSUM, partition_broadcast on the
    (otherwise idle) GPSIMD engine, one tensor_mul into bf16 ctx.
  - bv folded into the V projection via a ones-row rank-1 matmul.
  - software pipelining: scores(kt+1) is emitted before AV(kt) so the PE can
    run ahead of the ACT exp; next-chunk projections and prev-chunk output
    projections are interleaved into the attention loop as PE filler.
"""
import sys

import numpy as np

try:
    import concourse.bass as bass  # noqa: F401
except ImportError:
    sys.path.insert(0, "/opt/trn_rl_repo")

import concourse.bass as bass
import concourse.mybir as mybir
import concourse.tile as tile
from concourse import bacc
from concourse.bass_utils import run_bass_kernel_spmd

import ml_dtypes

FP32 = mybir.dt.float32
F32R = mybir.dt.float32r
BF16 = mybir.dt.bfloat16
AF = mybir.ActivationFunctionType
NPBF16 = ml_dtypes.bfloat16

B, S, D = 2, 2048, 1024
NH, DK = 16, 64
G = 4              # head groups (cores per batch)
HPG = NH // G      # heads per group = 4
NG = HPG * DK      # dims per group = 256
CH = 512           # q-chunk width
NCH = S // CH      # 4 chunks
NKT = S // 128     # 16 k-tiles
KD = D // 128      # 8 contraction tiles for projections
SCALE = 1.0 / np.sqrt(DK)
NEGBIG = -1.0e9

TRACE = False          # test harness can set kernel.TRACE = True
LAST_RESULTS = None    # test harness reads kernel.LAST_RESULTS
DEBUG = False          # dump intermediates to dbg_* outputs

_NC_CACHE = None


def _build_nc():
    nc = bacc.Bacc()
    xqT = nc.declare_dram_parameter("xqT", [D, S], BF16, isOutput=False)
    xkT = nc.declare_dram_parameter("xkT", [D, S], BF16, isOutput=False)
    xvT = nc.declare_dram_parameter("xvT", [D, S], BF16, isOutput=False)
    wq = nc.declare_dram_parameter("wq", [D, NG], BF16, isOutput=False)
    wk = nc.declare_dram_parameter("wk", [D, NG], BF16, isOutput=False)
    wv = nc.declare_dram_parameter("wv", [D, NG], BF16, isOutput=False)
    wo = nc.declare_dram_parameter("wo", [NG, D], BF16, isOutput=False)
    bq = nc.declare_dram_parameter("bq", [128, 2], FP32, isOutput=False)
    bk = nc.declare_dram_parameter("bk", [128, 2], FP32, isOutput=False)
    bvrow = nc.declare_dram_parameter("bvrow", [1, NG], BF16, isOutput=False)
    istrip = nc.declare_dram_parameter("istrip", [128, 128], BF16, isOutput=False)
    negid = nc.declare_dram_parameter("negid", [128, 128], BF16, isOutput=False)
    onesd = nc.declare_dram_parameter("onesd", [1, 128], BF16, isOutput=False)
    # bsel cols 0-127 = [1]*64 + [0]*64, cols 128-255 = [0]*64 + [1]*64:
    # K=1 matmul lhsTs that broadcast den_h0 to out partitions 0-63 and
    # den_h1 to 64-127
    bsel = nc.declare_dram_parameter("bsel", [1, 256], FP32, isOutput=False)
    # V lhsT template: zeros with the den-producing ones columns prefilled
    vtmpl = nc.declare_dram_parameter("vtmpl", [128, NKT * 2 * 256], BF16,
                                      isOutput=False)
    out = nc.declare_dram_parameter("out", [S, D], FP32, isOutput=True)
    if DEBUG:
        dbg = {
            "dbg_v": nc.declare_dram_parameter(
                "dbg_v", [128, NKT * 2 * 256], BF16, isOutput=True),
            "dbg_q": nc.declare_dram_parameter(
                "dbg_q", [128, 2 * S], BF16, isOutput=True),
            "dbg_k": nc.declare_dram_parameter(
                "dbg_k", [128, 2 * S], BF16, isOutput=True),
            "dbg_ctx": nc.declare_dram_parameter(
                "dbg_ctx", [128, 2 * S], BF16, isOutput=True),
            "dbg_rbbc": nc.declare_dram_parameter(
                "dbg_rbbc", [128, 8 * CH], FP32, isOutput=True),
            "dbg_pp": nc.declare_dram_parameter(
                "dbg_pp", [128, 2 * CH], BF16, isOutput=True),
        }

    with tile.TileContext(nc) as tc:
        with (
            tc.tile_pool(name="wpool", bufs=1) as wpool,
            tc.tile_pool(name="cpool", bufs=1) as cpool,
            tc.tile_pool(name="big", bufs=1) as big,
            tc.tile_pool(name="xq", bufs=2) as xqp,
            tc.tile_pool(name="xk", bufs=2) as xkp,
            tc.tile_pool(name="xv", bufs=2) as xvp,
            tc.tile_pool(name="pp", bufs=3) as ppool,
            tc.tile_pool(name="rb", bufs=2) as rbp,
            tc.tile_pool(name="rbbc", bufs=2) as rbbcp,
            tc.tile_pool(name="ot", bufs=2) as otp,
            tc.tile_pool(name="ps_s", bufs=2, space="PSUM") as ps_s,
            tc.tile_pool(name="ps_av", bufs=1, space="PSUM") as ps_av,
            tc.tile_pool(name="ps_w", bufs=2, space="PSUM") as ps_w,
        ):
            # ---- weights / constants (resident) ----
            wq_sb = wpool.tile([128, KD * NG], BF16, tag="wq")
            wk_sb = wpool.tile([128, KD * NG], BF16, tag="wk")
            wv_sb = wpool.tile([128, KD * NG], BF16, tag="wv")
            wo_sb = wpool.tile([128, 2 * D], BF16, tag="wo")
            for w_sb, w_dram in ((wq_sb, wq), (wk_sb, wk), (wv_sb, wv)):
                nc.sync.dma_start(
                    out=w_sb[:].rearrange("p (k n) -> p k n", k=KD),
                    in_=w_dram.rearrange("(k p) n -> p k n", p=128),
                )
            nc.sync.dma_start(
                out=wo_sb[:].rearrange("p (k m) -> p k m", k=2),
                in_=wo.rearrange("(k p) m -> p k m", p=128),
            )
            bq_sb = cpool.tile([128, 2], FP32, tag="bq")
            bk_sb = cpool.tile([128, 2], FP32, tag="bk")
            bv_sb = cpool.tile([1, NG], BF16, tag="bvrow")
            nc.sync.dma_start(out=bq_sb[:], in_=bq[:])
            nc.sync.dma_start(out=bk_sb[:], in_=bk[:])
            nc.sync.dma_start(out=bv_sb[:], in_=bvrow[:])
            istrip_sb = cpool.tile([128, 128], BF16, tag="istrip")
            negid_sb = cpool.tile([128, 128], BF16, tag="negid")
            nc.sync.dma_start(out=istrip_sb[:], in_=istrip[:])
            nc.sync.dma_start(out=negid_sb[:], in_=negid[:])
            ones_sb = cpool.tile([1, 128], BF16, tag="ones")
            nc.sync.dma_start(out=ones_sb[:], in_=onesd[:])
            bsel_sb = cpool.tile([1, 256], F32R, tag="bsel")
            nc.sync.dma_start(out=bsel_sb[:], in_=bsel[:, :].bitcast(F32R))

            # ---- persistent activations ----
            q_sb = [big.tile([128, S], BF16, tag=f"q{m}", name=f"q{m}") for m in range(2)]
            k_sb = [big.tile([128, S], BF16, tag=f"k{m}", name=f"k{m}") for m in range(2)]
            ctx_sb = [big.tile([128, S], BF16, tag=f"ctx{m}", name=f"ctx{m}")
                      for m in range(2)]
            # V lhsT blocks: [16 kt][2 pair][2 x 128 cols]; the zeros and the
            # den-producing ones columns come in via the DMA'd template
            v_sb = big.tile([128, NKT * 2 * 256], BF16, tag="v")
            nc.sync.dma_start(out=v_sb[:], in_=vtmpl[:])

            # ---------------- emission helpers ----------------
            x_tiles = {}

            def emit_x_dma(c):
                c0 = c * CH
                tiles = []
                for pool_, dram_ in ((xqp, xqT), (xkp, xkT), (xvp, xvT)):
                    t_ = pool_.tile([128, KD * CH], BF16, tag="x", name="xt")
                    nc.sync.dma_start(
                        out=t_[:].rearrange("p (k s) -> p k s", k=KD),
                        in_=dram_[:, c0:c0 + CH].rearrange("(k p) s -> p k s", p=128),
                    )
                    tiles.append(t_)
                x_tiles[c] = tiles

            def emit_qk_group(c, which, m):
                c0 = c * CH
                xt = x_tiles[c][0 if which == "q" else 1]
                w_sb_ = wq_sb if which == "q" else wk_sb
                dst = q_sb[m] if which == "q" else k_sb[m]
                b_sb_ = bq_sb if which == "q" else bk_sb
                pt = ps_w.tile([128, CH], FP32, tag="pw", name="pt")
                for kd in range(KD):
                    nc.tensor.matmul(
                        pt[:],
                        lhsT=w_sb_[:, kd * NG + m * 128: kd * NG + m * 128 + 128],
                        rhs=xt[:, kd * CH: kd * CH + CH],
                        start=(kd == 0), stop=(kd == KD - 1),
                    )
                nc.vector.tensor_scalar_add(dst[:, c0:c0 + CH], pt[:], b_sb_[:, m:m + 1])

            def emit_v_group(c, half):
                xt = x_tiles[c][2]
                pv = ps_w.tile([128, CH], FP32, tag="pw", name="pv")
                for sl in range(2):
                    ss = 2 * half + sl
                    col = sl * NG
                    for kd in range(KD):
                        nc.tensor.matmul(
                            pv[:, col:col + NG],
                            lhsT=xt[:, kd * CH + ss * 128: kd * CH + ss * 128 + 128],
                            rhs=wv_sb[:, kd * NG: kd * NG + NG],
                            start=(kd == 0), stop=False,
                        )
                    # fold bv in: out[s, n] += 1 * bv[n]
                    nc.tensor.matmul(
                        pv[:, col:col + NG],
                        lhsT=ones_sb[:],
                        rhs=bv_sb[:],
                        start=False, stop=True,
                    )
                for sl in range(2):
                    ss = 2 * half + sl
                    st = 4 * c + ss
                    col = sl * NG
                    src = pv[:, col:col + NG].rearrange("p (r i c) -> p r i c", r=2, i=2)
                    dst = v_sb[:].rearrange(
                        "p (t r a c) -> p t r a c", t=NKT, r=2, a=4
                    )[:, st, :, 0:4:3, :]
                    nc.vector.tensor_copy(dst, src)

            def emit_outproj_block(c, st):
                r0 = c * CH + st * 128
                ot = otp.tile([128, D], FP32, tag="ot", name="ot")
                for mo in range(2):
                    po = ps_w.tile([128, CH], FP32, tag="pw", name="po")
                    for kk in range(2):
                        nc.tensor.matmul(
                            po[:],
                            lhsT=ctx_sb[kk][:, r0:r0 + 128],
                            rhs=wo_sb[:, kk * D + mo * CH: kk * D + mo * CH + CH],
                            start=(kk == 0), stop=(kk == 1),
                        )
                    nc.vector.tensor_copy(ot[:, mo * CH: mo * CH + CH], po[:])
                nc.sync.dma_start(out=out[r0:r0 + 128, :], in_=ot[:])

            def emit_scores(c, mt, kt):
                c0 = c * CH
                j = kt - 4 * c
                w = CH - 128 * j if j > 0 else CH
                qo = c0 + (CH - w)
                sp = ps_s.tile([128, 2 * CH], FP32, tag="sp", name="sp")
                diag = j >= 0
                if not diag:
                    for i in range(2):
                        nc.tensor.matmul(
                            sp[:, i * CH: i * CH + w],
                            lhsT=k_sb[mt][64 * i: 64 * i + 64,
                                          kt * 128: kt * 128 + 128],
                            rhs=q_sb[mt][64 * i: 64 * i + 64, qo:qo + w],
                            start=True, stop=True,
                        )
                else:
                    # left 128 cols: scores + causal mask
                    # (scores[k, y] += -1e9 * strip[k, y] for y < k);
                    # right (w - 128) cols: plain scores
                    for i in range(2):
                        nc.tensor.matmul(
                            sp[:, i * CH: i * CH + 128],
                            lhsT=k_sb[mt][64 * i: 64 * i + 64,
                                          kt * 128: kt * 128 + 128],
                            rhs=q_sb[mt][64 * i: 64 * i + 64, qo:qo + 128],
                            start=True, stop=False,
                        )
                    for i in range(2):
                        nc.tensor.matmul(
                            sp[:, i * CH: i * CH + 128],
                            lhsT=negid_sb[:],
                            rhs=istrip_sb[:],
                            start=False, stop=True,
                        )
                    if w > 128:
                        for i in range(2):
                            nc.tensor.matmul(
                                sp[:, i * CH + 128: i * CH + w],
                                lhsT=k_sb[mt][64 * i: 64 * i + 64,
                                              kt * 128: kt * 128 + 128],
                                rhs=q_sb[mt][64 * i: 64 * i + 64,
                                             qo + 128:qo + w],
                                start=True, stop=True,
                            )
                return sp, w

            def emit_exp(sp, w):
                pp = ppool.tile([128, 2 * CH], BF16, tag="p", name="pp")
                sview = sp[:].rearrange("p (t x) -> p t x", t=2)[:, :, 0:w]
                pview = pp[:].rearrange("p (t x) -> p t x", t=2)[:, :, 0:w]
                nc.scalar.activation(pview, sview, AF.Exp, scale=SCALE)
                return pp

            def emit_av(c, mt, kt, av, pp, w, nkts):
                for i in range(2):
                    blk = (kt * 2 + mt) * 256 + i * 128
                    nc.tensor.matmul(
                        av[:, i * CH + (CH - w): i * CH + CH],
                        lhsT=v_sb[:, blk: blk + 128],
                        rhs=pp[:, i * CH: i * CH + w],
                        start=(kt == 0), stop=(kt == nkts - 1),
                    )

            def emit_normalize(c, mt, av):
                c0 = c * CH
                # h0 denominator at partition 64 of block0; h1 at partition 0
                # of block1. Copy to f32r (rounds - required by the verifier
                # for f32r matmul inputs), partition-broadcast via K=1
                # matmuls (DMA/GPSIMD broadcasts mis-handle a nonzero source
                # partition base on HW), then reciprocal the broadcast.
                denr = rbp.tile([1, 2 * CH], F32R, tag="rb", name="denr")
                nc.vector.tensor_copy(denr[0:1, 0:CH], av[64:65, 0:CH])
                nc.vector.tensor_copy(denr[0:1, CH:2 * CH], av[0:1, CH:2 * CH])
                pbc = ps_w.tile([128, CH], FP32, tag="pw", name="pbc")
                nc.tensor.matmul(
                    pbc[:], lhsT=bsel_sb[0:1, 0:128], rhs=denr[0:1, 0:CH],
                    start=True, stop=False)
                nc.tensor.matmul(
                    pbc[:], lhsT=bsel_sb[0:1, 128:256], rhs=denr[0:1, CH:2 * CH],
                    start=False, stop=True)
                rbbc = rbbcp.tile([128, CH], FP32, tag="rbbc", name="rbbc")
                nc.vector.reciprocal_approx_fast(out=rbbc[:], in_=pbc[:])
                if DEBUG:
                    idx = c * 2 + mt
                    nc.sync.dma_start(
                        out=dbg["dbg_rbbc"][:, idx * CH:(idx + 1) * CH],
                        in_=rbbc[:])
                nc.vector.tensor_mul(
                    ctx_sb[mt][0:64, c0:c0 + CH], av[0:64, 0:CH], rbbc[0:64, :])
                nc.vector.tensor_mul(
                    ctx_sb[mt][64:128, c0:c0 + CH],
                    av[64:128, CH:2 * CH], rbbc[64:128, :])

            # ---------------- main schedule ----------------
            emit_x_dma(0)
            emit_x_dma(1)
            emit_qk_group(0, "q", 0)
            emit_qk_group(0, "q", 1)
            emit_qk_group(0, "k", 0)
            emit_qk_group(0, "k", 1)
            emit_v_group(0, 0)
            emit_v_group(0, 1)

            for c in range(NCH):
                if c >= 1 and c + 1 < NCH:
                    emit_x_dma(c + 1)
                filler = []
                if c + 1 < NCH:
                    cc = c + 1
                    filler += [
                        lambda cc=cc: emit_qk_group(cc, "q", 0),
                        lambda cc=cc: emit_qk_group(cc, "q", 1),
                        lambda cc=cc: emit_qk_group(cc, "k", 0),
                        lambda cc=cc: emit_qk_group(cc, "k", 1),
                        lambda cc=cc: emit_v_group(cc, 0),
                        lambda cc=cc: emit_v_group(cc, 1),
                    ]
                if c >= 1:
                    cp = c - 1
                    filler += [
                        (lambda cp=cp, st=st: emit_outproj_block(cp, st))
                        for st in range(4)
                    ]

                nkts = 4 * c + 4
                n_iter = 2 * nkts
                stride = max(1, -(-n_iter // max(1, len(filler))))
                it = 0
                for mt in range(2):
                    av = ps_av.tile([128, 2 * CH], FP32, tag="av", name="av")
                    sp_w = emit_scores(c, mt, 0)
                    pending = sp_w
                    for kt in range(nkts):
                        if kt + 1 < nkts:
                            nxt = emit_scores(c, mt, kt + 1)
                        else:
                            nxt = None
                        pp = emit_exp(*pending)
                        if DEBUG and c == 0 and mt == 0 and kt == 0:
                            nc.sync.dma_start(out=dbg["dbg_pp"][:], in_=pp[:])
                        if filler and it % stride == 0:
                            filler.pop(0)()
                        emit_av(c, mt, kt, av, pp, pending[1], nkts)
                        pending = nxt
                        it += 1
                    emit_normalize(c, mt, av)
                while filler:
                    filler.pop(0)()

            for st in range(4):
                emit_outproj_block(NCH - 1, st)

            if DEBUG:
                nc.sync.dma_start(out=dbg["dbg_v"][:], in_=v_sb[:])
                for m in range(2):
                    nc.sync.dma_start(
                        out=dbg["dbg_q"][:, m * S:(m + 1) * S], in_=q_sb[m][:])
                    nc.sync.dma_start(
                        out=dbg["dbg_k"][:, m * S:(m + 1) * S], in_=k_sb[m][:])
                    nc.sync.dma_start(
                        out=dbg["dbg_ctx"][:, m * S:(m + 1) * S], in_=ctx_sb[m][:])

    nc.compile()
    return nc


def _get_nc():
    global _NC_CACHE
    if _NC_CACHE is None:
        _NC_CACHE = _build_nc()
    return _NC_CACHE


def _v_template() -> np.ndarray:
    # [128, NKT*2*256]: per (kt, pair): block0 = [V_h0 | one | 0*63],
    # block1 = [one | 0*63 | V_h1]; V columns are filled on-device, the
    # template provides the zeros and the ones columns
    t = np.zeros((128, NKT, 2, 256), np.float32)
    t[:, :, :, 64] = 1.0
    t[:, :, :, 128] = 1.0
    return t.reshape(128, NKT * 2 * 256).astype(NPBF16)


def _mask_strip_inv() -> np.ndarray:
    # istrip[p, y] = 1.0 iff y < p (masked); the causal q-offset makes every
    # diagonal k-tile's masked region fall in its first 128 columns
    y = np.arange(128)[None, :]
    p = np.arange(128)[:, None]
    return (y < p).astype(NPBF16)


def _reference_fallback(query, key, value, mask, wq, bq, wk, bk, wv, bv, wo, bo):
    out = np.empty((B, S, D), np.float32)
    for b in range(B):
        Q = (query[b] @ wq + bq).reshape(S, NH, DK).transpose(1, 0, 2)
        K = (key[b] @ wk + bk).reshape(S, NH, DK).transpose(1, 0, 2)
        V = (value[b] @ wv + bv).reshape(S, NH, DK).transpose(1, 0, 2)
        sc = np.einsum("hqd,hkd->hqk", Q, K).astype(np.float32) / np.sqrt(DK)
        sc = np.where(mask[b][None] == 0, -1.0e9, sc)
        sc -= sc.max(-1, keepdims=True)
        e = np.exp(sc)
        attn = e / e.sum(-1, keepdims=True)
        ctx = np.einsum("hqk,hkd->hqd", attn, V).transpose(1, 0, 2).reshape(S, D)
        out[b] = ctx @ wo + bo
    return out


def kernel(query, key, value, mask, wq, bq, wk, bk, wv, bv, wo, bo):
    global LAST_RESULTS
    query = np.asarray(query, np.float32)
    key = np.asarray(key, np.float32)
    value = np.asarray(value, np.float32)
    mask = np.asarray(mask)
    wq, bq = np.asarray(wq, np.float32), np.asarray(bq, np.float32)
    wk, bk = np.asarray(wk, np.float32), np.asarray(bk, np.float32)
    wv, bv = np.asarray(wv, np.float32), np.asarray(bv, np.float32)
    wo, bo = np.asarray(wo, np.float32), np.asarray(bo, np.float32)

    tril = np.tril(np.ones((S, S), mask.dtype))
    if not all(np.array_equal(mask[b], tril) for b in range(B)):
        return _reference_fallback(query, key, value, mask, wq, bq, wk, bk,
                                   wv, bv, wo, bo)

    istrip = _mask_strip_inv()
    negid = (np.eye(128, dtype=np.float32) * NEGBIG).astype(NPBF16)
    onesd = np.ones((1, 128), NPBF16)
    bselh = np.zeros((1, 256), np.float32)
    bselh[0, 0:64] = 1.0
    bselh[0, 192:256] = 1.0
    vtmpl = _v_template()
    xT = {}
    for b in range(B):
        xT[("q", b)] = np.ascontiguousarray(query[b].T).astype(NPBF16)
        xT[("k", b)] = np.ascontiguousarray(key[b].T).astype(NPBF16)
        xT[("v", b)] = np.ascontiguousarray(value[b].T).astype(NPBF16)

    in_maps = []
    for core in range(8):
        b, g = core // G, core % G
        cs = slice(g * NG, (g + 1) * NG)
        in_maps.append({
            "xqT": xT[("q", b)],
            "xkT": xT[("k", b)],
            "xvT": xT[("v", b)],
            "wq": np.ascontiguousarray(wq[:, cs]).astype(NPBF16),
            "wk": np.ascontiguousarray(wk[:, cs]).astype(NPBF16),
            "wv": np.ascontiguousarray(wv[:, cs]).astype(NPBF16),
            "wo": np.ascontiguousarray(wo[cs, :]).astype(NPBF16),
            "bq": np.ascontiguousarray(bq[cs].reshape(2, 128).T.astype(np.float32)),
            "bk": np.ascontiguousarray(bk[cs].reshape(2, 128).T.astype(np.float32)),
            "bvrow": bv[cs].reshape(1, NG).astype(NPBF16),
            "istrip": istrip,
            "negid": negid,
            "onesd": onesd,
            "bsel": bselh,
            "vtmpl": vtmpl,
        })

    nc = _get_nc()
    res = run_bass_kernel_spmd(nc, in_maps, list(range(8)), trace=TRACE)
    LAST_RESULTS = res

    out = np.empty((B, S, D), np.float32)
    for b in range(B):
        acc = res.results[b * G]["out"].astype(np.float32)
        for g in range(1, G):
            acc = acc + res.results[b * G + g]["out"]
        out[b] = acc + bo
    return out


# revision 43
# speedup vs baseline: 1.3248x; 1.0395x over previous
"""Multi-head attention (B=2, S=2048, D=1024, H=16 heads, causal) on 8 TRN2 cores.

Sharding: core i handles batch b=i//4 and head group g=i%4 (4 heads = 256 dims).
Each core computes QKV projections for its head group, causal attention, and a
partial output projection (its 256-dim slice of the contraction). Host sums the
4 partials per batch and adds the output bias.

v2 design (all matmuls bf16, fp32 PSUM accumulation):
  - Q^T, K^T: [n=256, s=2048] bf16 (head-pairs stacked on 128 partitions x 2)
  - V natural [s, n] bf16 per (s-tile, pair) as two 128-col lhsT blocks:
      block0 = [V_h0(64) | ones(1) | 0(63)]  -> AV out: ctx_h0 @ parts 0-63,
                                                den_h0 @ part 64
      block1 = [ones(1) | 0(63) | V_h1(64)]  -> AV out: den_h1 @ part 0,
                                                ctx_h1 @ parts 64-127
    so softmax denominators come out of the AV matmul for free AND the
    normalize tensor ops are lane-aligned with their ctx rows.
  - causal mask applied inside the scores PSUM via an extra matmul
    (diag(-1e9) @ strip[128,128]) -- only the first 128 cols of each diagonal
    tile can be masked, so N=128.
  - exp on ACT engine (scale=1/sqrt(dk)), output bf16 directly.
  - normalize: reciprocal_approx_fast from PSUM, partition_broadcast on the
    (otherwise idle) GPSIMD engine, one tensor_mul into bf16 ctx.
  - bv folded into the V projection via a ones-row rank-1 matmul.
  - software pipelining: scores(kt+1) is emitted before AV(kt) so the PE can
    run ahead of the ACT exp; next-chunk projections and prev-chunk output
    projections are interleaved into the attention loop as PE filler.
"""
import sys

import numpy as np

try:
    import concourse.bass as bass  # noqa: F401
except ImportError:
    sys.path.insert(0, "/opt/trn_rl_repo")

import concourse.bass as bass
import concourse.mybir as mybir
import concourse.tile as tile
from concourse import bacc
from concourse.bass_utils import run_bass_kernel_spmd

import ml_dtypes

FP32 = mybir.dt.float32
F32R = mybir.dt.float32r
BF16 = mybir.dt.bfloat16
AF = mybir.ActivationFunctionType
NPBF16 = ml_dtypes.bfloat16

B, S, D = 2, 2048, 1024
NH, DK = 16, 64
G = 4              # head groups (cores per batch)
HPG = NH // G      # heads per group = 4
NG = HPG * DK      # dims per group = 256
CH = 512           # q-chunk width
NCH = S // CH      # 4 chunks
NKT = S // 128     # 16 k-tiles
KD = D // 128      # 8 contraction tiles for projections
SCALE = 1.0 / np.sqrt(DK)
NEGBIG = -1.0e9

TRACE = False          # test harness can set kernel.TRACE = True
LAST_RESULTS = None    # test harness reads kernel.LAST_RESULTS
DEBUG = False          # dump intermediates to dbg_* outputs

_NC_CACHE = None


def _build_nc():
    nc = bacc.Bacc()
    xqT = nc.declare_dram_parameter("xqT", [D, S], BF16, isOutput=False)
    xkT = nc.declare_dram_parameter("xkT", [D, S], BF16, isOutput=False)
    xvT = nc.declare_dram_parameter("xvT", [D, S], BF16, isOutput=False)
    wq = nc.declare_dram_parameter("wq", [D, NG], BF16, isOutput=False)
    wk = nc.declare_dram_parameter("wk", [D, NG], BF16, isOutput=False)
    wv = nc.declare_dram_parameter("wv", [D, NG], BF16, isOutput=False)
    wo = nc.declare_dram_parameter("wo", [NG, D], BF16, isOutput=False)
    bq = nc.declare_dram_parameter("bq", [128, 2], FP32, isOutput=False)
    bk = nc.declare_dram_parameter("bk", [128, 2], FP32, isOutput=False)
    bvrow = nc.declare_dram_parameter("bvrow", [1, NG], BF16, isOutput=False)
    istrip = nc.declare_dram_parameter("istrip", [128, 128], BF16, isOutput=False)
    negid = nc.declare_dram_parameter("negid", [128, 128], BF16, isOutput=False)
    onesd = nc.declare_dram_parameter("onesd", [1, 128], BF16, isOutput=False)
    # bsel cols 0-127 = [1]*64 + [0]*64, cols 128-255 = [0]*64 + [1]*64:
    # K=1 matmul lhsTs that broadcast den_h0 to out partitions 0-63 and
    # den_h1 to 64-127
    bsel = nc.declare_dram_parameter("bsel", [1, 256], FP32, isOutput=False)
    # V lhsT template: zeros with the den-producing ones columns prefilled
    vtmpl = nc.declare_dram_parameter("vtmpl", [128, NKT * 2 * 256], BF16,
                                      isOutput=False)
    out = nc.declare_dram_parameter("out", [S, D], BF16, isOutput=True)
    if DEBUG:
        dbg = {
            "dbg_v": nc.declare_dram_parameter(
                "dbg_v", [128, NKT * 2 * 256], BF16, isOutput=True),
            "dbg_q": nc.declare_dram_parameter(
                "dbg_q", [128, 2 * S], BF16, isOutput=True),
            "dbg_k": nc.declare_dram_parameter(
                "dbg_k", [128, 2 * S], BF16, isOutput=True),
            "dbg_ctx": nc.declare_dram_parameter(
                "dbg_ctx", [128, 2 * S], BF16, isOutput=True),
            "dbg_rbbc": nc.declare_dram_parameter(
                "dbg_rbbc", [128, 8 * CH], FP32, isOutput=True),
            "dbg_pp": nc.declare_dram_parameter(
                "dbg_pp", [128, 2 * CH], BF16, isOutput=True),
        }

    with tile.TileContext(nc) as tc:
        with (
            tc.tile_pool(name="wpool", bufs=1) as wpool,
            tc.tile_pool(name="cpool", bufs=1) as cpool,
            tc.tile_pool(name="big", bufs=1) as big,
            tc.tile_pool(name="xq", bufs=2) as xqp,
            tc.tile_pool(name="xk", bufs=2) as xkp,
            tc.tile_pool(name="xv", bufs=2) as xvp,
            tc.tile_pool(name="pp", bufs=3) as ppool,
            tc.tile_pool(name="rb", bufs=2) as rbp,
            tc.tile_pool(name="rbbc", bufs=2) as rbbcp,
            tc.tile_pool(name="ot", bufs=2) as otp,
            tc.tile_pool(name="ps_s", bufs=2, space="PSUM") as ps_s,
            tc.tile_pool(name="ps_av", bufs=1, space="PSUM") as ps_av,
            tc.tile_pool(name="ps_w", bufs=2, space="PSUM") as ps_w,
        ):
            # ---- weights / constants (resident); DMAs are emitted in
            # critical-path order further below ----
            wq_sb = wpool.tile([128, KD * NG], BF16, tag="wq")
            wk_sb = wpool.tile([128, KD * NG], BF16, tag="wk")
            wv_sb = wpool.tile([128, KD * NG], BF16, tag="wv")
            wo_sb = wpool.tile([128, 2 * D], BF16, tag="wo")

            def dma_w(w_sb, w_dram):
                nc.sync.dma_start(
                    out=w_sb[:].rearrange("p (k n) -> p k n", k=KD),
                    in_=w_dram.rearrange("(k p) n -> p k n", p=128),
                )

            bq_sb = cpool.tile([128, 2], FP32, tag="bq")
            bk_sb = cpool.tile([128, 2], FP32, tag="bk")
            bv_sb = cpool.tile([1, NG], BF16, tag="bvrow")
            istrip_sb = cpool.tile([128, 128], BF16, tag="istrip")
            negid_sb = cpool.tile([128, 128], BF16, tag="negid")
            ones_sb = cpool.tile([1, 128], BF16, tag="ones")
            bsel_sb = cpool.tile([1, 256], F32R, tag="bsel")

            # ---- persistent activations ----
            q_sb = [big.tile([128, S], BF16, tag=f"q{m}", name=f"q{m}") for m in range(2)]
            k_sb = [big.tile([128, S], BF16, tag=f"k{m}", name=f"k{m}") for m in range(2)]
            ctx_sb = [big.tile([128, S], BF16, tag=f"ctx{m}", name=f"ctx{m}")
                      for m in range(2)]
            # V lhsT blocks: [16 kt][2 pair][2 x 128 cols]; the zeros and the
            # den-producing ones columns come in via the DMA'd template
            v_sb = big.tile([128, NKT * 2 * 256], BF16, tag="v")

            # ---------------- emission helpers ----------------
            x_tiles = {}
            _x_srcs = {"q": (xqp, xqT), "k": (xkp, xkT), "v": (xvp, xvT)}

            def emit_x1_dma(c, which, split=1):
                # split>1 emits multiple DMAs so the first proj matmuls can
                # start before the whole slice has landed
                c0 = c * CH
                pool_, dram_ = _x_srcs[which]
                t_ = pool_.tile([128, KD * CH], BF16, tag="x", name="xt")
                kper = KD // split
                for s in range(split):
                    k0 = s * kper
                    nc.sync.dma_start(
                        out=t_[:].rearrange("p (k s) -> p k s", k=KD)
                            [:, k0:k0 + kper, :],
                        in_=dram_[k0 * 128:(k0 + kper) * 128, c0:c0 + CH]
                            .rearrange("(k p) s -> p k s", p=128),
                    )
                x_tiles.setdefault(c, {})[which] = t_

            def emit_x_dma(c):
                for which in ("q", "k", "v"):
                    emit_x1_dma(c, which)

            def emit_qk_group(c, which, m):
                c0 = c * CH
                xt = x_tiles[c][which]
                w_sb_ = wq_sb if which == "q" else wk_sb
                dst = q_sb[m] if which == "q" else k_sb[m]
                b_sb_ = bq_sb if which == "q" else bk_sb
                pt = ps_w.tile([128, CH], FP32, tag="pw", name="pt")
                for kd in range(KD):
                    nc.tensor.matmul(
                        pt[:],
                        lhsT=w_sb_[:, kd * NG + m * 128: kd * NG + m * 128 + 128],
                        rhs=xt[:, kd * CH: kd * CH + CH],
                        start=(kd == 0), stop=(kd == KD - 1),
                    )
                nc.vector.tensor_scalar_add(dst[:, c0:c0 + CH], pt[:], b_sb_[:, m:m + 1])

            def emit_v_group(c, half):
                xt = x_tiles[c]["v"]
                pv = ps_w.tile([128, CH], FP32, tag="pw", name="pv")
                for sl in range(2):
                    ss = 2 * half + sl
                    col = sl * NG
                    for kd in range(KD):
                        nc.tensor.matmul(
                            pv[:, col:col + NG],
                            lhsT=xt[:, kd * CH + ss * 128: kd * CH + ss * 128 + 128],
                            rhs=wv_sb[:, kd * NG: kd * NG + NG],
                            start=(kd == 0), stop=False,
                        )
                    # fold bv in: out[s, n] += 1 * bv[n]
                    nc.tensor.matmul(
                        pv[:, col:col + NG],
                        lhsT=ones_sb[:],
                        rhs=bv_sb[:],
                        start=False, stop=True,
                    )
                for sl in range(2):
                    ss = 2 * half + sl
                    st = 4 * c + ss
                    col = sl * NG
                    src = pv[:, col:col + NG].rearrange("p (r i c) -> p r i c", r=2, i=2)
                    dst = v_sb[:].rearrange(
                        "p (t r a c) -> p t r a c", t=NKT, r=2, a=4
                    )[:, st, :, 0:4:3, :]
                    nc.vector.tensor_copy(dst, src)

            def emit_outproj_block(c, st):
                r0 = c * CH + st * 128
                ot = otp.tile([128, D], BF16, tag="ot", name="ot")
                for mo in range(2):
                    po = ps_w.tile([128, CH], FP32, tag="pw", name="po")
                    for kk in range(2):
                        nc.tensor.matmul(
                            po[:],
                            lhsT=ctx_sb[kk][:, r0:r0 + 128],
                            rhs=wo_sb[:, kk * D + mo * CH: kk * D + mo * CH + CH],
                            start=(kk == 0), stop=(kk == 1),
                        )
                    nc.vector.tensor_copy(ot[:, mo * CH: mo * CH + CH], po[:])
                nc.sync.dma_start(out=out[r0:r0 + 128, :], in_=ot[:])

            def emit_scores(c, mt, kt):
                c0 = c * CH
                j = kt - 4 * c
                w = CH - 128 * j if j > 0 else CH
                qo = c0 + (CH - w)
                sp = ps_s.tile([128, 2 * CH], FP32, tag="sp", name="sp")
                diag = j >= 0
                if not diag:
                    for i in range(2):
                        nc.tensor.matmul(
                            sp[:, i * CH: i * CH + w],
                            lhsT=k_sb[mt][64 * i: 64 * i + 64,
                                          kt * 128: kt * 128 + 128],
                            rhs=q_sb[mt][64 * i: 64 * i + 64, qo:qo + w],
                            start=True, stop=True,
                        )
                else:
                    # left 128 cols: scores + causal mask
                    # (scores[k, y] += -1e9 * strip[k, y] for y < k);
                    # right (w - 128) cols: plain scores
                    for i in range(2):
                        nc.tensor.matmul(
                            sp[:, i * CH: i * CH + 128],
                            lhsT=k_sb[mt][64 * i: 64 * i + 64,
                                          kt * 128: kt * 128 + 128],
                            rhs=q_sb[mt][64 * i: 64 * i + 64, qo:qo + 128],
                            start=True, stop=False,
                        )
                    for i in range(2):
                        nc.tensor.matmul(
                            sp[:, i * CH: i * CH + 128],
                            lhsT=negid_sb[:],
                            rhs=istrip_sb[:],
                            start=False, stop=True,
                        )
                    if w > 128:
                        for i in range(2):
                            nc.tensor.matmul(
                                sp[:, i * CH + 128: i * CH + w],
                                lhsT=k_sb[mt][64 * i: 64 * i + 64,
                                              kt * 128: kt * 128 + 128],
                                rhs=q_sb[mt][64 * i: 64 * i + 64,
                                             qo + 128:qo + w],
                                start=True, stop=True,
                            )
                return sp, w

            def emit_exp(sp, w):
                pp = ppool.tile([128, 2 * CH], BF16, tag="p", name="pp")
                sview = sp[:].rearrange("p (t x) -> p t x", t=2)[:, :, 0:w]
                pview = pp[:].rearrange("p (t x) -> p t x", t=2)[:, :, 0:w]
                nc.scalar.activation(pview, sview, AF.Exp, scale=SCALE)
                return pp

            def emit_av(c, mt, kt, av, pp, w, nkts):
                for i in range(2):
                    blk = (kt * 2 + mt) * 256 + i * 128
                    nc.tensor.matmul(
                        av[:, i * CH + (CH - w): i * CH + CH],
                        lhsT=v_sb[:, blk: blk + 128],
                        rhs=pp[:, i * CH: i * CH + w],
                        start=(kt == 0), stop=(kt == nkts - 1),
                    )

            def emit_normalize(c, mt, av):
                c0 = c * CH
                # h0 denominator at partition 64 of block0; h1 at partition 0
                # of block1. Copy to f32r (rounds - required by the verifier
                # for f32r matmul inputs), partition-broadcast via K=1
                # matmuls (DMA/GPSIMD broadcasts mis-handle a nonzero source
                # partition base on HW), then reciprocal the broadcast.
                denr = rbp.tile([1, 2 * CH], F32R, tag="rb", name="denr")
                nc.vector.tensor_copy(denr[0:1, 0:CH], av[64:65, 0:CH])
                nc.vector.tensor_copy(denr[0:1, CH:2 * CH], av[0:1, CH:2 * CH])
                pbc = ps_w.tile([128, CH], FP32, tag="pw", name="pbc")
                nc.tensor.matmul(
                    pbc[:], lhsT=bsel_sb[0:1, 0:128], rhs=denr[0:1, 0:CH],
                    start=True, stop=False)
                nc.tensor.matmul(
                    pbc[:], lhsT=bsel_sb[0:1, 128:256], rhs=denr[0:1, CH:2 * CH],
                    start=False, stop=True)
                rbbc = rbbcp.tile([128, CH], FP32, tag="rbbc", name="rbbc")
                nc.vector.reciprocal_approx_fast(out=rbbc[:], in_=pbc[:])
                if DEBUG:
                    idx = c * 2 + mt
                    nc.sync.dma_start(
                        out=dbg["dbg_rbbc"][:, idx * CH:(idx + 1) * CH],
                        in_=rbbc[:])
                nc.vector.tensor_mul(
                    ctx_sb[mt][0:64, c0:c0 + CH], av[0:64, 0:CH], rbbc[0:64, :])
                nc.vector.tensor_mul(
                    ctx_sb[mt][64:128, c0:c0 + CH],
                    av[64:128, CH:2 * CH], rbbc[64:128, :])

            # ---------------- main schedule ----------------
            # startup DMAs in critical-path order: Q-proj inputs first so the
            # PE starts ~8us in, everything else behind them on the queue
            dma_w(wq_sb, wq)
            nc.sync.dma_start(out=bq_sb[:], in_=bq[:])
            emit_x1_dma(0, "q", split=2)
            emit_qk_group(0, "q", 0)
            emit_qk_group(0, "q", 1)
            dma_w(wk_sb, wk)
            nc.sync.dma_start(out=bk_sb[:], in_=bk[:])
            emit_x1_dma(0, "k", split=2)
            emit_qk_group(0, "k", 0)
            emit_qk_group(0, "k", 1)
            dma_w(wv_sb, wv)
            nc.sync.dma_start(out=bv_sb[:], in_=bvrow[:])
            nc.sync.dma_start(out=ones_sb[:], in_=onesd[:])
            emit_x1_dma(0, "v", split=2)
            nc.sync.dma_start(out=v_sb[:], in_=vtmpl[:])
            emit_v_group(0, 0)
            emit_v_group(0, 1)
            nc.sync.dma_start(out=istrip_sb[:], in_=istrip[:])
            nc.sync.dma_start(out=negid_sb[:], in_=negid[:])
            nc.sync.dma_start(out=bsel_sb[:], in_=bsel[:, :].bitcast(F32R))
            nc.sync.dma_start(
                out=wo_sb[:].rearrange("p (k m) -> p k m", k=2),
                in_=wo.rearrange("(k p) m -> p k m", p=128),
            )
            emit_x_dma(1)

            for c in range(NCH):
                if c >= 1 and c + 1 < NCH:
                    emit_x_dma(c + 1)
                filler = []
                if c + 1 < NCH:
                    cc = c + 1
                    filler += [
                        lambda cc=cc: emit_qk_group(cc, "q", 0),
                        lambda cc=cc: emit_qk_group(cc, "q", 1),
                        lambda cc=cc: emit_qk_group(cc, "k", 0),
                        lambda cc=cc: emit_qk_group(cc, "k", 1),
                        lambda cc=cc: emit_v_group(cc, 0),
                        lambda cc=cc: emit_v_group(cc, 1),
                    ]
                if c >= 1:
                    cp = c - 1
                    filler += [
                        (lambda cp=cp, st=st: emit_outproj_block(cp, st))
                        for st in range(4)
                    ]

                nkts = 4 * c + 4
                n_iter = 2 * nkts
                stride = max(1, -(-n_iter // max(1, len(filler))))
                it = 0
                for mt in range(2):
                    av = ps_av.tile([128, 2 * CH], FP32, tag="av", name="av")
                    sp_w = emit_scores(c, mt, 0)
                    pending = sp_w
                    for kt in range(nkts):
                        if kt + 1 < nkts:
                            nxt = emit_scores(c, mt, kt + 1)
                        else:
                            nxt = None
                        pp = emit_exp(*pending)
                        if DEBUG and c == 0 and mt == 0 and kt == 0:
                            nc.sync.dma_start(out=dbg["dbg_pp"][:], in_=pp[:])
                        if filler and it % stride == 0:
                            filler.pop(0)()
                        emit_av(c, mt, kt, av, pp, pending[1], nkts)
                        pending = nxt
                        it += 1
                    emit_normalize(c, mt, av)
                while filler:
                    filler.pop(0)()

            for st in range(4):
                emit_outproj_block(NCH - 1, st)

            if DEBUG:
                nc.sync.dma_start(out=dbg["dbg_v"][:], in_=v_sb[:])
                for m in range(2):
                    nc.sync.dma_start(
                        out=dbg["dbg_q"][:, m * S:(m + 1) * S], in_=q_sb[m][:])
                    nc.sync.dma_start(
                        out=dbg["dbg_k"][:, m * S:(m + 1) * S], in_=k_sb[m][:])
                    nc.sync.dma_start(
                        out=dbg["dbg_ctx"][:, m * S:(m + 1) * S], in_=ctx_sb[m][:])

    nc.compile()
    return nc


def _get_nc():
    global _NC_CACHE
    if _NC_CACHE is None:
        _NC_CACHE = _build_nc()
    return _NC_CACHE


def _v_template() -> np.ndarray:
    # [128, NKT*2*256]: per (kt, pair): block0 = [V_h0 | one | 0*63],
    # block1 = [one | 0*63 | V_h1]; V columns are filled on-device, the
    # template provides the zeros and the ones columns
    t = np.zeros((128, NKT, 2, 256), np.float32)
    t[:, :, :, 64] = 1.0
    t[:, :, :, 128] = 1.0
    return t.reshape(128, NKT * 2 * 256).astype(NPBF16)


def _mask_strip_inv() -> np.ndarray:
    # istrip[p, y] = 1.0 iff y < p (masked); the causal q-offset makes every
    # diagonal k-tile's masked region fall in its first 128 columns
    y = np.arange(128)[None, :]
    p = np.arange(128)[:, None]
    return (y < p).astype(NPBF16)


def _reference_fallback(query, key, value, mask, wq, bq, wk, bk, wv, bv, wo, bo):
    out = np.empty((B, S, D), np.float32)
    for b in range(B):
        Q = (query[b] @ wq + bq).reshape(S, NH, DK).transpose(1, 0, 2)
        K = (key[b] @ wk + bk).reshape(S, NH, DK).transpose(1, 0, 2)
        V = (value[b] @ wv + bv).reshape(S, NH, DK).transpose(1, 0, 2)
        sc = np.einsum("hqd,hkd->hqk", Q, K).astype(np.float32) / np.sqrt(DK)
        sc = np.where(mask[b][None] == 0, -1.0e9, sc)
        sc -= sc.max(-1, keepdims=True)
        e = np.exp(sc)
        attn = e / e.sum(-1, keepdims=True)
        ctx = np.einsum("hqk,hkd->hqd", attn, V).transpose(1, 0, 2).reshape(S, D)
        out[b] = ctx @ wo + bo
    return out


def kernel(query, key, value, mask, wq, bq, wk, bk, wv, bv, wo, bo):
    global LAST_RESULTS
    query = np.asarray(query, np.float32)
    key = np.asarray(key, np.float32)
    value = np.asarray(value, np.float32)
    mask = np.asarray(mask)
    wq, bq = np.asarray(wq, np.float32), np.asarray(bq, np.float32)
    wk, bk = np.asarray(wk, np.float32), np.asarray(bk, np.float32)
    wv, bv = np.asarray(wv, np.float32), np.asarray(bv, np.float32)
    wo, bo = np.asarray(wo, np.float32), np.asarray(bo, np.float32)

    tril = np.tril(np.ones((S, S), mask.dtype))
    if not all(np.array_equal(mask[b], tril) for b in range(B)):
        return _reference_fallback(query, key, value, mask, wq, bq, wk, bk,
                                   wv, bv, wo, bo)

    istrip = _mask_strip_inv()
    negid = (np.eye(128, dtype=np.float32) * NEGBIG).astype(NPBF16)
    onesd = np.ones((1, 128), NPBF16)
    bselh = np.zeros((1, 256), np.float32)
    bselh[0, 0:64] = 1.0
    bselh[0, 192:256] = 1.0
    vtmpl = _v_template()
    xT = {}
    for b in range(B):
        xT[("q", b)] = np.ascontiguousarray(query[b].T).astype(NPBF16)
        xT[("k", b)] = np.ascontiguousarray(key[b].T).astype(NPBF16)
        xT[("v", b)] = np.ascontiguousarray(value[b].T).astype(NPBF16)

    in_maps = []
    for core in range(8):
        b, g = core // G, core % G
        cs = slice(g * NG, (g + 1) * NG)
        in_maps.append({
            "xqT": xT[("q", b)],
            "xkT": xT[("k", b)],
            "xvT": xT[("v", b)],
            "wq": np.ascontiguousarray(wq[:, cs]).astype(NPBF16),
            "wk": np.ascontiguousarray(wk[:, cs]).astype(NPBF16),
            "wv": np.ascontiguousarray(wv[:, cs]).astype(NPBF16),
            "wo": np.ascontiguousarray(wo[cs, :]).astype(NPBF16),
            "bq": np.ascontiguousarray(bq[cs].reshape(2, 128).T.astype(np.float32)),
            "bk": np.ascontiguousarray(bk[cs].reshape(2, 128).T.astype(np.float32)),
            "bvrow": bv[cs].reshape(1, NG).astype(NPBF16),
            "istrip": istrip,
            "negid": negid,
            "onesd": onesd,
            "bsel": bselh,
            "vtmpl": vtmpl,
        })

    nc = _get_nc()
    res = run_bass_kernel_spmd(nc, in_maps, list(range(8)), trace=TRACE)
    LAST_RESULTS = res

    out = np.empty((B, S, D), np.float32)
    for b in range(B):
        acc = res.results[b * G]["out"].astype(np.float32)
        for g in range(1, G):
            acc = acc + res.results[b * G + g]["out"]
        out[b] = acc + bo
    return out


# revision 46
# speedup vs baseline: 1.3401x; 1.0115x over previous
"""Multi-head attention (B=2, S=2048, D=1024, H=16 heads, causal) on 8 TRN2 cores.

Sharding: core i handles batch b=i//4 and head group g=i%4 (4 heads = 256 dims).
Each core computes QKV projections for its head group, causal attention, and a
partial output projection (its 256-dim slice of the contraction). Host sums the
4 partials per batch and adds the output bias.

v2 design (all matmuls bf16, fp32 PSUM accumulation):
  - Q^T, K^T: [n=256, s=2048] bf16 (head-pairs stacked on 128 partitions x 2)
  - V natural [s, n] bf16 per (s-tile, pair) as two 128-col lhsT blocks:
      block0 = [V_h0(64) | ones(1) | 0(63)]  -> AV out: ctx_h0 @ parts 0-63,
                                                den_h0 @ part 64
      block1 = [ones(1) | 0(63) | V_h1(64)]  -> AV out: den_h1 @ part 0,
                                                ctx_h1 @ parts 64-127
    so softmax denominators come out of the AV matmul for free AND the
    normalize tensor ops are lane-aligned with their ctx rows.
  - causal mask applied inside the scores PSUM via an extra matmul
    (diag(-1e9) @ strip[128,128]) -- only the first 128 cols of each diagonal
    tile can be masked, so N=128.
  - exp on ACT engine (scale=1/sqrt(dk)), output bf16 directly.
  - normalize: reciprocal_approx_fast from PSUM, partition_broadcast on the
    (otherwise idle) GPSIMD engine, one tensor_mul into bf16 ctx.
  - bv folded into the V projection via a ones-row rank-1 matmul.
  - software pipelining: scores(kt+1) is emitted before AV(kt) so the PE can
    run ahead of the ACT exp; next-chunk projections and prev-chunk output
    projections are interleaved into the attention loop as PE filler.
"""
import sys

import numpy as np

try:
    import concourse.bass as bass  # noqa: F401
except ImportError:
    sys.path.insert(0, "/opt/trn_rl_repo")

import concourse.bass as bass
import concourse.mybir as mybir
import concourse.tile as tile
from concourse import bacc
from concourse.bass_utils import run_bass_kernel_spmd

import ml_dtypes

FP32 = mybir.dt.float32
F32R = mybir.dt.float32r
BF16 = mybir.dt.bfloat16
AF = mybir.ActivationFunctionType
NPBF16 = ml_dtypes.bfloat16

B, S, D = 2, 2048, 1024
NH, DK = 16, 64
G = 4              # head groups (cores per batch)
HPG = NH // G      # heads per group = 4
NG = HPG * DK      # dims per group = 256
CH = 512           # q-chunk width
NCH = S // CH      # 4 chunks
NKT = S // 128     # 16 k-tiles
KD = D // 128      # 8 contraction tiles for projections
SCALE = 1.0 / np.sqrt(DK)
NEGBIG = -1.0e9

TRACE = False          # test harness can set kernel.TRACE = True
LAST_RESULTS = None    # test harness reads kernel.LAST_RESULTS
DEBUG = False          # dump intermediates to dbg_* outputs

_NC_CACHE = None


def _build_nc():
    nc = bacc.Bacc()
    xqT = nc.declare_dram_parameter("xqT", [D, S], BF16, isOutput=False)
    xkT = nc.declare_dram_parameter("xkT", [D, S], BF16, isOutput=False)
    xvT = nc.declare_dram_parameter("xvT", [D, S], BF16, isOutput=False)
    wq = nc.declare_dram_parameter("wq", [D, NG], BF16, isOutput=False)
    wk = nc.declare_dram_parameter("wk", [D, NG], BF16, isOutput=False)
    wv = nc.declare_dram_parameter("wv", [D, NG], BF16, isOutput=False)
    wo = nc.declare_dram_parameter("wo", [NG, D], BF16, isOutput=False)
    bq = nc.declare_dram_parameter("bq", [128, 2], FP32, isOutput=False)
    bk = nc.declare_dram_parameter("bk", [128, 2], FP32, isOutput=False)
    bvrow = nc.declare_dram_parameter("bvrow", [1, NG], BF16, isOutput=False)
    istrip = nc.declare_dram_parameter("istrip", [128, 128], BF16, isOutput=False)
    negid = nc.declare_dram_parameter("negid", [128, 128], BF16, isOutput=False)
    onesd = nc.declare_dram_parameter("onesd", [1, 128], BF16, isOutput=False)
    # bsel cols 0-127 = [1]*64 + [0]*64, cols 128-255 = [0]*64 + [1]*64:
    # K=1 matmul lhsTs that broadcast den_h0 to out partitions 0-63 and
    # den_h1 to 64-127
    bsel = nc.declare_dram_parameter("bsel", [1, 256], FP32, isOutput=False)
    # V lhsT template: zeros with the den-producing ones columns prefilled
    vtmpl = nc.declare_dram_parameter("vtmpl", [128, NKT * 2 * 256], BF16,
                                      isOutput=False)
    out = nc.declare_dram_parameter("out", [S, D], BF16, isOutput=True)
    if DEBUG:
        dbg = {
            "dbg_v": nc.declare_dram_parameter(
                "dbg_v", [128, NKT * 2 * 256], BF16, isOutput=True),
            "dbg_q": nc.declare_dram_parameter(
                "dbg_q", [128, 2 * S], BF16, isOutput=True),
            "dbg_k": nc.declare_dram_parameter(
                "dbg_k", [128, 2 * S], BF16, isOutput=True),
            "dbg_ctx": nc.declare_dram_parameter(
                "dbg_ctx", [128, 2 * S], BF16, isOutput=True),
            "dbg_rbbc": nc.declare_dram_parameter(
                "dbg_rbbc", [128, 8 * CH], FP32, isOutput=True),
            "dbg_pp": nc.declare_dram_parameter(
                "dbg_pp", [128, 2 * CH], BF16, isOutput=True),
        }

    with tile.TileContext(nc) as tc:
        with (
            tc.tile_pool(name="wpool", bufs=1) as wpool,
            tc.tile_pool(name="cpool", bufs=1) as cpool,
            tc.tile_pool(name="big", bufs=1) as big,
            tc.tile_pool(name="xq", bufs=2) as xqp,
            tc.tile_pool(name="xk", bufs=2) as xkp,
            tc.tile_pool(name="xv", bufs=2) as xvp,
            tc.tile_pool(name="pp", bufs=3) as ppool,
            tc.tile_pool(name="rb", bufs=2) as rbp,
            tc.tile_pool(name="rbbc", bufs=2) as rbbcp,
            tc.tile_pool(name="ot", bufs=2) as otp,
            tc.tile_pool(name="ps_s", bufs=2, space="PSUM") as ps_s,
            tc.tile_pool(name="ps_av", bufs=1, space="PSUM") as ps_av,
            tc.tile_pool(name="ps_w", bufs=2, space="PSUM") as ps_w,
        ):
            # ---- weights / constants (resident); DMAs are emitted in
            # critical-path order further below ----
            wq_sb = wpool.tile([128, KD * NG], BF16, tag="wq")
            wk_sb = wpool.tile([128, KD * NG], BF16, tag="wk")
            wv_sb = wpool.tile([128, KD * NG], BF16, tag="wv")
            wo_sb = wpool.tile([128, 2 * D], BF16, tag="wo")

            def dma_w(w_sb, w_dram):
                nc.sync.dma_start(
                    out=w_sb[:].rearrange("p (k n) -> p k n", k=KD),
                    in_=w_dram.rearrange("(k p) n -> p k n", p=128),
                )

            bq_sb = cpool.tile([128, 2], FP32, tag="bq")
            bk_sb = cpool.tile([128, 2], FP32, tag="bk")
            bv_sb = cpool.tile([1, NG], BF16, tag="bvrow")
            istrip_sb = cpool.tile([128, 128], BF16, tag="istrip")
            negid_sb = cpool.tile([128, 128], BF16, tag="negid")
            ones_sb = cpool.tile([1, 128], BF16, tag="ones")
            bsel_sb = cpool.tile([1, 256], F32R, tag="bsel")

            # ---- persistent activations ----
            q_sb = [big.tile([128, S], BF16, tag=f"q{m}", name=f"q{m}") for m in range(2)]
            k_sb = [big.tile([128, S], BF16, tag=f"k{m}", name=f"k{m}") for m in range(2)]
            ctx_sb = [big.tile([128, S], BF16, tag=f"ctx{m}", name=f"ctx{m}")
                      for m in range(2)]
            # V lhsT blocks: [16 kt][2 pair][2 x 128 cols]; the zeros and the
            # den-producing ones columns come in via the DMA'd template
            v_sb = big.tile([128, NKT * 2 * 256], BF16, tag="v")

            # ---------------- emission helpers ----------------
            x_tiles = {}
            _x_srcs = {"q": (xqp, xqT), "k": (xkp, xkT), "v": (xvp, xvT)}

            def emit_x1_dma(c, which, split=1):
                # split>1 emits multiple DMAs so the first proj matmuls can
                # start before the whole slice has landed
                c0 = c * CH
                pool_, dram_ = _x_srcs[which]
                t_ = pool_.tile([128, KD * CH], BF16, tag="x", name="xt")
                kper = KD // split
                for s in range(split):
                    k0 = s * kper
                    nc.sync.dma_start(
                        out=t_[:].rearrange("p (k s) -> p k s", k=KD)
                            [:, k0:k0 + kper, :],
                        in_=dram_[k0 * 128:(k0 + kper) * 128, c0:c0 + CH]
                            .rearrange("(k p) s -> p k s", p=128),
                    )
                x_tiles.setdefault(c, {})[which] = t_

            def emit_x_dma(c):
                for which in ("q", "k", "v"):
                    emit_x1_dma(c, which)

            def emit_qk_group(c, which, m):
                c0 = c * CH
                xt = x_tiles[c][which]
                w_sb_ = wq_sb if which == "q" else wk_sb
                dst = q_sb[m] if which == "q" else k_sb[m]
                b_sb_ = bq_sb if which == "q" else bk_sb
                pt = ps_w.tile([128, CH], FP32, tag="pw", name="pt")
                for kd in range(KD):
                    nc.tensor.matmul(
                        pt[:],
                        lhsT=w_sb_[:, kd * NG + m * 128: kd * NG + m * 128 + 128],
                        rhs=xt[:, kd * CH: kd * CH + CH],
                        start=(kd == 0), stop=(kd == KD - 1),
                    )
                nc.vector.tensor_scalar_add(dst[:, c0:c0 + CH], pt[:], b_sb_[:, m:m + 1])

            def emit_v_group(c, half):
                xt = x_tiles[c]["v"]
                pv = ps_w.tile([128, CH], FP32, tag="pw", name="pv")
                for sl in range(2):
                    ss = 2 * half + sl
                    col = sl * NG
                    for kd in range(KD):
                        nc.tensor.matmul(
                            pv[:, col:col + NG],
                            lhsT=xt[:, kd * CH + ss * 128: kd * CH + ss * 128 + 128],
                            rhs=wv_sb[:, kd * NG: kd * NG + NG],
                            start=(kd == 0), stop=False,
                        )
                    # fold bv in: out[s, n] += 1 * bv[n]
                    nc.tensor.matmul(
                        pv[:, col:col + NG],
                        lhsT=ones_sb[:],
                        rhs=bv_sb[:],
                        start=False, stop=True,
                    )
                for sl in range(2):
                    ss = 2 * half + sl
                    st = 4 * c + ss
                    col = sl * NG
                    src = pv[:, col:col + NG].rearrange("p (r i c) -> p r i c", r=2, i=2)
                    dst = v_sb[:].rearrange(
                        "p (t r a c) -> p t r a c", t=NKT, r=2, a=4
                    )[:, st, :, 0:4:3, :]
                    nc.vector.tensor_copy(dst, src)

            def emit_outproj_block(c, st, act_copy=False):
                r0 = c * CH + st * 128
                ot = otp.tile([128, D], BF16, tag="ot", name="ot")
                for mo in range(2):
                    po = ps_w.tile([128, CH], FP32, tag="pw", name="po")
                    for kk in range(2):
                        nc.tensor.matmul(
                            po[:],
                            lhsT=ctx_sb[kk][:, r0:r0 + 128],
                            rhs=wo_sb[:, kk * D + mo * CH: kk * D + mo * CH + CH],
                            start=(kk == 0), stop=(kk == 1),
                        )
                    # at the drain tail ACT is idle: split the PSUM drains
                    # across both engines
                    if act_copy and mo == 1:
                        nc.scalar.copy(ot[:, mo * CH: mo * CH + CH], po[:])
                    else:
                        nc.vector.tensor_copy(ot[:, mo * CH: mo * CH + CH], po[:])
                nc.sync.dma_start(out=out[r0:r0 + 128, :], in_=ot[:])

            def emit_scores(c, mt, kt):
                c0 = c * CH
                j = kt - 4 * c
                w = CH - 128 * j if j > 0 else CH
                qo = c0 + (CH - w)
                sp = ps_s.tile([128, 2 * CH], FP32, tag="sp", name="sp")
                diag = j >= 0
                if not diag:
                    for i in range(2):
                        nc.tensor.matmul(
                            sp[:, i * CH: i * CH + w],
                            lhsT=k_sb[mt][64 * i: 64 * i + 64,
                                          kt * 128: kt * 128 + 128],
                            rhs=q_sb[mt][64 * i: 64 * i + 64, qo:qo + w],
                            start=True, stop=True,
                            tile_position=(64 * i, 0),
                        )
                else:
                    # left 128 cols: scores + causal mask
                    # (scores[k, y] += -1e9 * strip[k, y] for y < k);
                    # right (w - 128) cols: plain scores
                    for i in range(2):
                        nc.tensor.matmul(
                            sp[:, i * CH: i * CH + 128],
                            lhsT=k_sb[mt][64 * i: 64 * i + 64,
                                          kt * 128: kt * 128 + 128],
                            rhs=q_sb[mt][64 * i: 64 * i + 64, qo:qo + 128],
                            start=True, stop=False,
                            tile_position=(64 * i, 0),
                        )
                    for i in range(2):
                        nc.tensor.matmul(
                            sp[:, i * CH: i * CH + 128],
                            lhsT=negid_sb[:],
                            rhs=istrip_sb[:],
                            start=False, stop=True,
                        )
                    if w > 128:
                        for i in range(2):
                            nc.tensor.matmul(
                                sp[:, i * CH + 128: i * CH + w],
                                lhsT=k_sb[mt][64 * i: 64 * i + 64,
                                              kt * 128: kt * 128 + 128],
                                rhs=q_sb[mt][64 * i: 64 * i + 64,
                                             qo + 128:qo + w],
                                start=True, stop=True,
                                tile_position=(64 * i, 0),
                            )
                return sp, w

            def emit_exp(sp, w):
                pp = ppool.tile([128, 2 * CH], BF16, tag="p", name="pp")
                sview = sp[:].rearrange("p (t x) -> p t x", t=2)[:, :, 0:w]
                pview = pp[:].rearrange("p (t x) -> p t x", t=2)[:, :, 0:w]
                nc.scalar.activation(pview, sview, AF.Exp, scale=SCALE)
                return pp

            def emit_av(c, mt, kt, av, pp, w, nkts):
                for i in range(2):
                    blk = (kt * 2 + mt) * 256 + i * 128
                    nc.tensor.matmul(
                        av[:, i * CH + (CH - w): i * CH + CH],
                        lhsT=v_sb[:, blk: blk + 128],
                        rhs=pp[:, i * CH: i * CH + w],
                        start=(kt == 0), stop=(kt == nkts - 1),
                    )

            def emit_normalize(c, mt, av):
                c0 = c * CH
                # h0 denominator at partition 64 of block0; h1 at partition 0
                # of block1. Copy to f32r (rounds - required by the verifier
                # for f32r matmul inputs), partition-broadcast via K=1
                # matmuls (DMA/GPSIMD broadcasts mis-handle a nonzero source
                # partition base on HW), then reciprocal the broadcast.
                denr = rbp.tile([1, 2 * CH], F32R, tag="rb", name="denr")
                nc.vector.tensor_copy(denr[0:1, 0:CH], av[64:65, 0:CH])
                nc.vector.tensor_copy(denr[0:1, CH:2 * CH], av[0:1, CH:2 * CH])
                pbc = ps_w.tile([128, CH], FP32, tag="pw", name="pbc")
                nc.tensor.matmul(
                    pbc[:], lhsT=bsel_sb[0:1, 0:128], rhs=denr[0:1, 0:CH],
                    start=True, stop=False)
                nc.tensor.matmul(
                    pbc[:], lhsT=bsel_sb[0:1, 128:256], rhs=denr[0:1, CH:2 * CH],
                    start=False, stop=True)
                rbbc = rbbcp.tile([128, CH], FP32, tag="rbbc", name="rbbc")
                nc.vector.reciprocal_approx_fast(out=rbbc[:], in_=pbc[:])
                if DEBUG:
                    idx = c * 2 + mt
                    nc.sync.dma_start(
                        out=dbg["dbg_rbbc"][:, idx * CH:(idx + 1) * CH],
                        in_=rbbc[:])
                nc.vector.tensor_mul(
                    ctx_sb[mt][0:64, c0:c0 + CH], av[0:64, 0:CH], rbbc[0:64, :])
                nc.vector.tensor_mul(
                    ctx_sb[mt][64:128, c0:c0 + CH],
                    av[64:128, CH:2 * CH], rbbc[64:128, :])

            # ---------------- main schedule ----------------
            # startup DMAs in critical-path order: Q-proj inputs first so the
            # PE starts ~8us in, everything else behind them on the queue
            dma_w(wq_sb, wq)
            nc.sync.dma_start(out=bq_sb[:], in_=bq[:])
            emit_x1_dma(0, "q", split=2)
            emit_qk_group(0, "q", 0)
            emit_qk_group(0, "q", 1)
            dma_w(wk_sb, wk)
            nc.sync.dma_start(out=bk_sb[:], in_=bk[:])
            emit_x1_dma(0, "k", split=2)
            emit_qk_group(0, "k", 0)
            emit_qk_group(0, "k", 1)
            dma_w(wv_sb, wv)
            nc.sync.dma_start(out=bv_sb[:], in_=bvrow[:])
            nc.sync.dma_start(out=ones_sb[:], in_=onesd[:])
            emit_x1_dma(0, "v", split=2)
            nc.sync.dma_start(out=v_sb[:], in_=vtmpl[:])
            emit_v_group(0, 0)
            emit_v_group(0, 1)
            nc.sync.dma_start(out=istrip_sb[:], in_=istrip[:])
            nc.sync.dma_start(out=negid_sb[:], in_=negid[:])
            nc.sync.dma_start(out=bsel_sb[:], in_=bsel[:, :].bitcast(F32R))
            nc.sync.dma_start(
                out=wo_sb[:].rearrange("p (k m) -> p k m", k=2),
                in_=wo.rearrange("(k p) m -> p k m", p=128),
            )
            emit_x_dma(1)

            for c in range(NCH):
                if c >= 1 and c + 1 < NCH:
                    emit_x_dma(c + 1)
                filler = []
                if c + 1 < NCH:
                    cc = c + 1
                    filler += [
                        lambda cc=cc: emit_qk_group(cc, "q", 0),
                        lambda cc=cc: emit_qk_group(cc, "q", 1),
                        lambda cc=cc: emit_qk_group(cc, "k", 0),
                        lambda cc=cc: emit_qk_group(cc, "k", 1),
                        lambda cc=cc: emit_v_group(cc, 0),
                        lambda cc=cc: emit_v_group(cc, 1),
                    ]
                if c >= 1:
                    cp = c - 1
                    filler += [
                        (lambda cp=cp, st=st: emit_outproj_block(cp, st))
                        for st in range(4)
                    ]

                nkts = 4 * c + 4
                n_iter = 2 * nkts
                stride = max(1, -(-n_iter // max(1, len(filler))))
                it = 0
                for mt in range(2):
                    av = ps_av.tile([128, 2 * CH], FP32, tag="av", name="av")
                    sp_w = emit_scores(c, mt, 0)
                    pending = sp_w
                    for kt in range(nkts):
                        if kt + 1 < nkts:
                            nxt = emit_scores(c, mt, kt + 1)
                        else:
                            nxt = None
                        pp = emit_exp(*pending)
                        if DEBUG and c == 0 and mt == 0 and kt == 0:
                            nc.sync.dma_start(out=dbg["dbg_pp"][:], in_=pp[:])
                        if filler and it % stride == 0:
                            filler.pop(0)()
                        emit_av(c, mt, kt, av, pp, pending[1], nkts)
                        pending = nxt
                        it += 1
                    emit_normalize(c, mt, av)
                while filler:
                    filler.pop(0)()

            for st in range(4):
                emit_outproj_block(NCH - 1, st, act_copy=True)

            if DEBUG:
                nc.sync.dma_start(out=dbg["dbg_v"][:], in_=v_sb[:])
                for m in range(2):
                    nc.sync.dma_start(
                        out=dbg["dbg_q"][:, m * S:(m + 1) * S], in_=q_sb[m][:])
                    nc.sync.dma_start(
                        out=dbg["dbg_k"][:, m * S:(m + 1) * S], in_=k_sb[m][:])
                    nc.sync.dma_start(
                        out=dbg["dbg_ctx"][:, m * S:(m + 1) * S], in_=ctx_sb[m][:])

    nc.compile()
    return nc


def _get_nc():
    global _NC_CACHE
    if _NC_CACHE is None:
        _NC_CACHE = _build_nc()
    return _NC_CACHE


def _v_template() -> np.ndarray:
    # [128, NKT*2*256]: per (kt, pair): block0 = [V_h0 | one | 0*63],
    # block1 = [one | 0*63 | V_h1]; V columns are filled on-device, the
    # template provides the zeros and the ones columns
    t = np.zeros((128, NKT, 2, 256), np.float32)
    t[:, :, :, 64] = 1.0
    t[:, :, :, 128] = 1.0
    return t.reshape(128, NKT * 2 * 256).astype(NPBF16)


def _mask_strip_inv() -> np.ndarray:
    # istrip[p, y] = 1.0 iff y < p (masked); the causal q-offset makes every
    # diagonal k-tile's masked region fall in its first 128 columns
    y = np.arange(128)[None, :]
    p = np.arange(128)[:, None]
    return (y < p).astype(NPBF16)


def _reference_fallback(query, key, value, mask, wq, bq, wk, bk, wv, bv, wo, bo):
    out = np.empty((B, S, D), np.float32)
    for b in range(B):
        Q = (query[b] @ wq + bq).reshape(S, NH, DK).transpose(1, 0, 2)
        K = (key[b] @ wk + bk).reshape(S, NH, DK).transpose(1, 0, 2)
        V = (value[b] @ wv + bv).reshape(S, NH, DK).transpose(1, 0, 2)
        sc = np.einsum("hqd,hkd->hqk", Q, K).astype(np.float32) / np.sqrt(DK)
        sc = np.where(mask[b][None] == 0, -1.0e9, sc)
        sc -= sc.max(-1, keepdims=True)
        e = np.exp(sc)
        attn = e / e.sum(-1, keepdims=True)
        ctx = np.einsum("hqk,hkd->hqd", attn, V).transpose(1, 0, 2).reshape(S, D)
        out[b] = ctx @ wo + bo
    return out


def kernel(query, key, value, mask, wq, bq, wk, bk, wv, bv, wo, bo):
    global LAST_RESULTS
    query = np.asarray(query, np.float32)
    key = np.asarray(key, np.float32)
    value = np.asarray(value, np.float32)
    mask = np.asarray(mask)
    wq, bq = np.asarray(wq, np.float32), np.asarray(bq, np.float32)
    wk, bk = np.asarray(wk, np.float32), np.asarray(bk, np.float32)
    wv, bv = np.asarray(wv, np.float32), np.asarray(bv, np.float32)
    wo, bo = np.asarray(wo, np.float32), np.asarray(bo, np.float32)

    tril = np.tril(np.ones((S, S), mask.dtype))
    if not all(np.array_equal(mask[b], tril) for b in range(B)):
        return _reference_fallback(query, key, value, mask, wq, bq, wk, bk,
                                   wv, bv, wo, bo)

    istrip = _mask_strip_inv()
    negid = (np.eye(128, dtype=np.float32) * NEGBIG).astype(NPBF16)
    onesd = np.ones((1, 128), NPBF16)
    bselh = np.zeros((1, 256), np.float32)
    bselh[0, 0:64] = 1.0
    bselh[0, 192:256] = 1.0
    vtmpl = _v_template()
    xT = {}
    for b in range(B):
        xT[("q", b)] = np.ascontiguousarray(query[b].T).astype(NPBF16)
        xT[("k", b)] = np.ascontiguousarray(key[b].T).astype(NPBF16)
        xT[("v", b)] = np.ascontiguousarray(value[b].T).astype(NPBF16)

    in_maps = []
    for core in range(8):
        b, g = core // G, core % G
        cs = slice(g * NG, (g + 1) * NG)
        in_maps.append({
            "xqT": xT[("q", b)],
            "xkT": xT[("k", b)],
            "xvT": xT[("v", b)],
            "wq": np.ascontiguousarray(wq[:, cs]).astype(NPBF16),
            "wk": np.ascontiguousarray(wk[:, cs]).astype(NPBF16),
            "wv": np.ascontiguousarray(wv[:, cs]).astype(NPBF16),
            "wo": np.ascontiguousarray(wo[cs, :]).astype(NPBF16),
            "bq": np.ascontiguousarray(bq[cs].reshape(2, 128).T.astype(np.float32)),
            "bk": np.ascontiguousarray(bk[cs].reshape(2, 128).T.astype(np.float32)),
            "bvrow": bv[cs].reshape(1, NG).astype(NPBF16),
            "istrip": istrip,
            "negid": negid,
            "onesd": onesd,
            "bsel": bselh,
            "vtmpl": vtmpl,
        })

    nc = _get_nc()
    res = run_bass_kernel_spmd(nc, in_maps, list(range(8)), trace=TRACE)
    LAST_RESULTS = res

    out = np.empty((B, S, D), np.float32)
    for b in range(B):
        acc = res.results[b * G]["out"].astype(np.float32)
        for g in range(1, G):
            acc = acc + res.results[b * G + g]["out"]
        out[b] = acc + bo
    return out
